# revision 11
# baseline (speedup 1.0000x reference)
"""Sliding-window GQA attention (RoPE + qk-RMSNorm + sink logits) on 8 trn2 cores.

Sharding: tensor-parallel over heads. Core c owns q-heads {2c, 2c+1} and kv-head
c//2 (duplicated across core pairs).  Each core computes its heads' attention and
a partial output projection [S, D]; a ReduceScatter sums partials and hands core
c rows [256c, 256c+256) of the final output, which the host concatenates.

Layout strategy on-core:
  - x is loaded naturally [s, d] and transposed on PE (identity matmul) to
    xT [d, s] tiles for the q/kv projections (contract over d).
  - q/k/v are produced naturally [s, f]; clip, RMSNorm (ACT square+accum) and
    RoPE (DVE, sign/weight folded into host cos/sin tables) run in that layout.
  - q/k are PE-transposed to [hd, s]; scores are computed transposed
    [k_s, q] so the softmax denominator is a ones-column matmul (PSUM [q, 1]-free
    orientation avoided entirely), exp runs on ACT with the 1/sqrt(HD) scale
    fused, no max-subtraction (RMSNorm bounds |logits| <= sqrt(HD)).
  - Sliding window (1024) + causal mask = at most 10 k-tiles per 256-row q
    block, with only 4 precomputed [128, 256] masks (band edges).
  - PV gives attT [hd, q]; 1/denom is broadcast via a rank-1 PE outer product
    and applied during the attT PSUM->SBUF copy; out-projection accumulates
    both heads in PSUM and DMAs straight to the DRAM partial buffer.
All matmuls run as float32r (full f32 storage, 1 cycle/row at free dim >= 256).
"""

import numpy as np

import concourse.bass as bass
import concourse.bacc as bacc
import concourse.mybir as mybir
from concourse import tile
from concourse.bass_utils import run_bass_kernel_spmd

F32 = mybir.dt.float32
F32R = mybir.dt.float32r

S = 2048
DM = 2048
H = 16
KVH = 4
HD = 128
WINDOW = 1024
CLIP = 8.0
THETA = 500000.0
EPS = 1e-6
SCALE = 1.0 / float(np.sqrt(HD))
NCORE = 8
SSH = S // NCORE          # 256 rows of final output per core
QB = 256                  # q block (2 s-tiles)
NQB = S // QB             # 8 blocks
NT = S // 128             # 16 s-tiles
MASKVAL = -1.0e4

# module-level knobs for test harness
TRACE = False
LAST_EXEC_NS = None
LAST_RESULT = None

_cached_nc = None


def r(ap):
    """view an AP as float32r for full-rate PE matmuls"""
    return ap.bitcast(F32R)


def build_graph() -> bass.Bass:
    nc = bacc.Bacc("TRN2", target_bir_lowering=False, debug=False)

    x_ext = nc.declare_dram_parameter("x", [S, DM], F32, isOutput=False)
    wq_ext = nc.declare_dram_parameter("wq", [2 * HD, DM], F32, isOutput=False)
    wkv_ext = nc.declare_dram_parameter("wkv", [2 * HD, DM], F32, isOutput=False)
    wout_ext = nc.declare_dram_parameter("wout", [DM, 2 * HD], F32, isOutput=False)
    cosq_ext = nc.declare_dram_parameter("cosq", [S, HD], F32, isOutput=False)
    sinq_ext = nc.declare_dram_parameter("sinq", [S, HD], F32, isOutput=False)
    cosk_ext = nc.declare_dram_parameter("cosk", [S, HD], F32, isOutput=False)
    sink_ext = nc.declare_dram_parameter("sink", [S, HD], F32, isOutput=False)
    masks_ext = nc.declare_dram_parameter("masks", [4, 128, QB], F32, isOutput=False)
    sinkexp_ext = nc.declare_dram_parameter("sinkexp", [1, 2], F32, isOutput=False)
    ident_ext = nc.declare_dram_parameter("ident", [128, 128], F32, isOutput=False)
    out_ext = nc.declare_dram_parameter("out", [SSH, DM], F32, isOutput=True)

    with tile.TileContext(nc) as tc:
        with (
            tc.tile_pool(name="const", bufs=1) as const,
            tc.tile_pool(name="persist", bufs=1) as persist,
            tc.tile_pool(name="dram", bufs=1, space="DRAM") as dram,
        ):
            ident = const.tile([128, 128], F32, tag="ident")
            nc.sync.dma_start(out=ident[:], in_=ident_ext[:])
            masks = const.tile([128, 4 * QB], F32, tag="masks")
            for i in range(4):
                nc.sync.dma_start(
                    out=masks[:, i * QB : (i + 1) * QB], in_=masks_ext[i]
                )
            sinkexp = const.tile([1, 2], F32, tag="sinkexp")
            nc.sync.dma_start(out=sinkexp[:], in_=sinkexp_ext[:])
            ones_f32 = const.tile([128, 1], F32, tag="ones_f32")
            nc.vector.memset(ones_f32[:], 1.0)
            ones_col = const.tile([128, 1], F32R, tag="ones_col")
            nc.vector.tensor_copy(ones_col[:], ones_f32[:])
            ones_row = const.tile([1, 128], F32, tag="ones_row")
            nc.vector.memset(ones_row[:], 1.0)

            # rope tables, natural layout: tbl[p, st*128+j] = T[st*128+p, j]
            tbls = {}
            for name, ext in (
                ("cosq", cosq_ext),
                ("sinq", sinq_ext),
                ("cosk", cosk_ext),
                ("sink", sink_ext),
            ):
                t = const.tile([128, S], F32, tag=f"tbl_{name}")
                nc.sync.dma_start(
                    out=t[:].rearrange("p (st j) -> p st j", st=NT),
                    in_=ext.rearrange("(st p) j -> p st j", p=128),
                )
                tbls[name] = t

            # transposed weights, built via PE transposes of natural loads
            wqT = persist.tile([128, 16 * 256], F32R, tag="wqT")
            wkvT = persist.tile([128, 16 * 256], F32R, tag="wkvT")
            woutT = persist.tile([128, 2 * DM], F32R, tag="woutT")

            with (
                tc.tile_pool(name="wstage", bufs=2) as wstage,
                tc.tile_pool(name="wtp", bufs=4, space="PSUM") as wtp,
            ):
                for wext, wT in ((wq_ext, wqT), (wkv_ext, wkvT)):
                    for f in range(2):
                        stg = wstage.tile([128, DM], F32, tag="wstg")
                        nc.sync.dma_start(
                            out=stg[:], in_=wext[f * 128 : (f + 1) * 128, :]
                        )
                        for kd in range(16):
                            pt = wtp.tile([128, 128], F32, tag="wtp")
                            nc.tensor.transpose(
                                pt[:], stg[:, kd * 128 : (kd + 1) * 128], ident[:]
                            )
                            nc.scalar.copy(
                                wT[:, kd * 256 + f * 128 : kd * 256 + f * 128 + 128],
                                pt[:],
                            )
                for jt in range(16):
                    stg = wstage.tile([128, 2 * HD], F32, tag="wostg")
                    nc.sync.dma_start(
                        out=stg[:], in_=wout_ext[jt * 128 : (jt + 1) * 128, :]
                    )
                    for h in range(2):
                        pt = wtp.tile([128, 128], F32, tag="wtp")
                        nc.tensor.transpose(
                            pt[:], stg[:, h * 128 : (h + 1) * 128], ident[:]
                        )
                        nc.scalar.copy(
                            woutT[:, h * DM + jt * 128 : h * DM + jt * 128 + 128],
                            pt[:],
                        )

            # persistent per-core attention tensors
            qT = persist.tile([128, 2 * S], F32R, tag="qT")    # [hd, h*S + s]
            kT = persist.tile([128, S], F32R, tag="kT")        # [hd, s]
            v_sb = persist.tile([128, S], F32R, tag="v_sb")    # [s-tile part, hd]

            # ---------------- phase 1: projections + norm + rope ----------
            with (
                tc.tile_pool(name="xpool", bufs=2) as xpool,
                tc.tile_pool(name="xtsb", bufs=2) as xtsb,
                tc.tile_pool(name="p1ps", bufs=2, space="PSUM") as p1ps,
                tc.tile_pool(name="projps", bufs=2, space="PSUM") as projps,
                tc.tile_pool(name="natp", bufs=3) as natp,
                tc.tile_pool(name="smal", bufs=8) as smal,
                tc.tile_pool(name="ropep", bufs=4) as ropep,
            ):
                for st in range(NT):
                    xt = xpool.tile([128, DM], F32, tag="x_nat")
                    nc.sync.dma_start(
                        out=xt[:], in_=x_ext[st * 128 : (st + 1) * 128, :]
                    )
                    xTb = xtsb.tile([128, DM], F32R, tag="xT")
                    for kd in range(16):
                        pt = p1ps.tile([128, 128], F32, tag="xt_ps")
                        nc.tensor.transpose(
                            pt[:], xt[:, kd * 128 : (kd + 1) * 128], ident[:]
                        )
                        nc.scalar.copy(xTb[:, kd * 128 : (kd + 1) * 128], pt[:])

                    psq = projps.tile([128, 256], F32, tag="psq")
                    pskv = projps.tile([128, 256], F32, tag="pskv")
                    for kd in range(16):
                        lhs = xTb[:, kd * 128 : (kd + 1) * 128]
                        nc.tensor.matmul(
                            psq[:],
                            r(lhs),
                            r(wqT[:, kd * 256 : (kd + 1) * 256]),
                            start=(kd == 0),
                            stop=(kd == 15),
                        )
                    for kd in range(16):
                        lhs = xTb[:, kd * 128 : (kd + 1) * 128]
                        nc.tensor.matmul(
                            pskv[:],
                            r(lhs),
                            r(wkvT[:, kd * 256 : (kd + 1) * 256]),
                            start=(kd == 0),
                            stop=(kd == 15),
                        )

                    # clip
                    qc = natp.tile([128, 256], F32, tag="qc")
                    nc.vector.tensor_scalar(
                        qc[:], psq[:], -CLIP, CLIP,
                        mybir.AluOpType.max, mybir.AluOpType.min,
                    )
                    kc = natp.tile([128, 128], F32, tag="kc")
                    nc.vector.tensor_scalar(
                        kc[:], pskv[:, 0:128], -CLIP, CLIP,
                        mybir.AluOpType.max, mybir.AluOpType.min,
                    )
                    # v straight to persistent (clipped)
                    nc.vector.tensor_scalar(
                        v_sb[:, st * 128 : (st + 1) * 128], pskv[:, 128:256],
                        -CLIP, CLIP,
                        mybir.AluOpType.max, mybir.AluOpType.min,
                    )

                    # rmsnorm + rope for q heads and k head
                    for which, src, cos_t, sin_t in (
                        ("q0", qc[:, 0:128], tbls["cosq"], tbls["sinq"]),
                        ("q1", qc[:, 128:256], tbls["cosq"], tbls["sinq"]),
                        ("k", kc[:], tbls["cosk"], tbls["sink"]),
                    ):
                        scr = smal.tile([128, 128], F32, tag="sq_scr")
                        ssq = smal.tile([128, 1], F32, tag="ssq")
                        nc.scalar.activation(
                            scr[:], src, mybir.ActivationFunctionType.Square,
                            accum_out=ssq[:],
                        )
                        varv = smal.tile([128, 1], F32, tag="varv")
                        nc.vector.tensor_scalar(
                            varv[:], ssq[:], 1.0 / HD, EPS,
                            mybir.AluOpType.mult, mybir.AluOpType.add,
                        )
                        sdv = smal.tile([128, 1], F32, tag="sdv")
                        nc.scalar.sqrt(sdv[:], varv[:])
                        rms = smal.tile([128, 1], F32, tag="rms")
                        nc.vector.reciprocal(rms[:], sdv[:])

                        cos_s = cos_t[:, st * 128 : (st + 1) * 128]
                        sin_s = sin_t[:, st * 128 : (st + 1) * 128]
                        rp = ropep.tile([128, 128], F32, tag="rope")
                        tmp = ropep.tile([128, 128], F32, tag="ropetmp")
                        # tmp = (x_shifted * rms) * sin_tbl   (sign in table)
                        nc.vector.scalar_tensor_tensor(
                            tmp[:, 0:64], src[:, 64:128], rms[:], sin_s[:, 0:64],
                            mybir.AluOpType.mult, mybir.AluOpType.mult,
                        )
                        nc.vector.scalar_tensor_tensor(
                            tmp[:, 64:128], src[:, 0:64], rms[:], sin_s[:, 64:128],
                            mybir.AluOpType.mult, mybir.AluOpType.mult,
                        )
                        # rp = (x * rms) * cos_tbl + tmp
                        nc.vector.scalar_tensor_tensor(
                            rp[:], src, rms[:], cos_s,
                            mybir.AluOpType.mult, mybir.AluOpType.mult,
                        )
                        nc.vector.tensor_tensor(
                            rp[:], rp[:], tmp[:], mybir.AluOpType.add
                        )

                        # transpose to [hd, s]
                        pt = p1ps.tile([128, 128], F32, tag="qkt_ps")
                        nc.tensor.transpose(pt[:], rp[:], ident[:])
                        if which == "q0":
                            dst = qT[:, st * 128 : (st + 1) * 128]
                        elif which == "q1":
                            dst = qT[:, S + st * 128 : S + (st + 1) * 128]
                        else:
                            dst = kT[:, st * 128 : (st + 1) * 128]
                        nc.scalar.copy(dst, pt[:])

            # ---------------- phase 2: attention + out projection ---------
            partial = dram.tile([S, DM], F32, tag="partial")
            with (
                tc.tile_pool(name="p256", bufs=2, space="PSUM") as p256,
                tc.tile_pool(name="attps", bufs=2, space="PSUM") as attps,
                tc.tile_pool(name="denps", bufs=2, space="PSUM") as denps,
                tc.tile_pool(name="pops", bufs=2, space="PSUM") as pops,
                tc.tile_pool(name="ptsb", bufs=4) as ptsb,
                tc.tile_pool(name="otsb", bufs=4) as otsb,
                tc.tile_pool(name="ostg", bufs=3) as ostg,
                tc.tile_pool(name="blksm", bufs=8) as blksm,
            ):
                for ts in range(NQB):
                    ktg0 = max(0, 2 * ts - 8)
                    nk = 2 * ts + 2 - ktg0
                    att_ps = [
                        attps.tile([128, QB], F32, tag="att_ps", name=f"att_ps_{ts}_{h}")
                        for h in range(2)
                    ]
                    den = [
                        denps.tile([1, QB], F32, tag="den", name=f"den_{ts}_{h}")
                        for h in range(2)
                    ]
                    for ikt in range(nk):
                        ktg = ktg0 + ikt
                        kk = kT[:, ktg * 128 : (ktg + 1) * 128]
                        vv = v_sb[:, ktg * 128 : (ktg + 1) * 128]
                        # mask index for this k tile (band edges), else -1
                        mi = -1
                        if ts >= 4 and ikt == 0:
                            mi = 0
                        elif ts >= 4 and ikt == 1:
                            mi = 1
                        elif ikt == nk - 2:
                            mi = 2
                        elif ikt == nk - 1:
                            mi = 3
                        for h in range(2):
                            qs = qT[:, h * S + ts * QB : h * S + (ts + 1) * QB]
                            sc = p256.tile([128, QB], F32, tag="score_ps")
                            nc.tensor.matmul(
                                sc[:], r(kk), r(qs), start=True, stop=True
                            )
                            if mi >= 0:
                                nc.vector.tensor_tensor(
                                    sc[:], sc[:],
                                    masks[:, mi * QB : (mi + 1) * QB],
                                    mybir.AluOpType.add,
                                )
                            pT = ptsb.tile([128, QB], F32R, tag="pT")
                            nc.scalar.activation(
                                pT[:], sc[:], mybir.ActivationFunctionType.Exp,
                                scale=SCALE,
                            )
                            nc.tensor.matmul(
                                den[h][:], r(ones_col[:]), r(pT[:]),
                                start=(ikt == 0), stop=(ikt == nk - 1),
                            )
                            nc.tensor.matmul(
                                att_ps[h][:], r(vv), r(pT[:]),
                                start=(ikt == 0), stop=(ikt == nk - 1),
                            )

                    oT = []
                    for h in range(2):
                        dsb = blksm.tile([1, QB], F32, tag="den_sb")
                        nc.vector.tensor_scalar(
                            dsb[:], den[h][:],
                            sinkexp[0:1, h : h + 1], None,
                            mybir.AluOpType.add,
                        )
                        rec = blksm.tile([1, QB], F32, tag="recip")
                        nc.vector.reciprocal(rec[:], dsb[:])
                        rbp = p256.tile([128, QB], F32, tag="score_ps", name=f"rb_ps_{ts}_{h}")
                        nc.tensor.matmul(
                            rbp[:], ones_row[:], rec[:],
                            start=True, stop=True,
                        )
                        rbs = blksm.tile([128, QB], F32, tag="rb_sb")
                        nc.scalar.copy(rbs[:], rbp[:])
                        ot = otsb.tile([128, QB], F32R, tag="oT")
                        nc.vector.tensor_tensor(
                            ot[:], att_ps[h][:], rbs[:], mybir.AluOpType.mult
                        )
                        oT.append(ot)

                    for stl in range(2):
                        srow = ts * QB + stl * 128
                        for jt in range(4):
                            po = pops.tile([128, 512], F32, tag="po")
                            for h in range(2):
                                nc.tensor.matmul(
                                    po[:],
                                    r(oT[h][:, stl * 128 : (stl + 1) * 128]),
                                    r(woutT[:, h * DM + jt * 512 : h * DM + (jt + 1) * 512]),
                                    start=(h == 0), stop=(h == 1),
                                )
                            og = ostg.tile([128, 512], F32, tag="og")
                            nc.scalar.copy(og[:], po[:])
                            nc.sync.dma_start(
                                out=partial[srow : srow + 128, jt * 512 : (jt + 1) * 512],
                                in_=og[:],
                            )

            # ---------------- phase 3: reduce-scatter + output ------------
            rs_out = dram.tile([SSH, DM], F32, tag="rs_out")
            nc.gpsimd.collective_compute(
                "ReduceScatter",
                mybir.AluOpType.add,
                replica_groups=[list(range(NCORE))],
                ins=[partial.opt()],
                outs=[rs_out.opt()],
            )
            nc.sync.dma_start(out=out_ext[:], in_=rs_out[:])

    nc.compile()
    return nc


def host_tables(q_norm_w, k_norm_w):
    inv_freq = 1.0 / (THETA ** (np.arange(0, HD, 2, dtype=np.float64) / HD))
    pos = np.arange(S, dtype=np.float64)
    freqs = pos[:, None] * inv_freq[None, :]
    emb = np.concatenate([freqs, freqs], axis=-1)
    cos = np.cos(emb)
    sin = np.sin(emb)
    sgn = np.concatenate([-np.ones(HD // 2), np.ones(HD // 2)])
    out = {}
    for pfx, w in (("q", q_norm_w), ("k", k_norm_w)):
        w = w.astype(np.float64)
        wshift = np.concatenate([w[HD // 2 :], w[: HD // 2]])
        out["cos" + pfx] = (cos * w[None, :]).astype(np.float32)
        out["sin" + pfx] = (sin * wshift[None, :] * sgn[None, :]).astype(np.float32)
    return out


def host_masks():
    p = np.arange(128)[:, None]
    f = np.arange(128)[None, :]
    strict_lower = np.where(p > f, 0.0, MASKVAL).astype(np.float32)   # p > f valid
    incl_upper = np.where(p <= f, 0.0, MASKVAL).astype(np.float32)    # p <= f valid
    full = np.full((128, 128), MASKVAL, dtype=np.float32)
    zero = np.zeros((128, 128), dtype=np.float32)
    mA = np.concatenate([strict_lower, full], axis=1)
    mB = np.concatenate([zero, strict_lower], axis=1)
    mC = np.concatenate([incl_upper, zero], axis=1)
    mD = np.concatenate([full, incl_upper], axis=1)
    return np.stack([mA, mB, mC, mD])


def kernel(x, w_q, w_k, w_v, w_out, q_norm_w, k_norm_w, sinks):
    global _cached_nc, LAST_EXEC_NS, LAST_RESULT
    x2 = np.ascontiguousarray(x.reshape(S, DM).astype(np.float32))
    tbl = host_tables(np.asarray(q_norm_w), np.asarray(k_norm_w))
    masks = host_masks()
    ident = np.eye(128, dtype=np.float32)

    in_maps = []
    for c in range(NCORE):
        g = c // 2
        wq_c = np.ascontiguousarray(w_q[2 * HD * c : 2 * HD * (c + 1), :]).astype(np.float32)
        wkv_c = np.concatenate(
            [w_k[HD * g : HD * (g + 1), :], w_v[HD * g : HD * (g + 1), :]], axis=0
        ).astype(np.float32)
        wout_c = np.ascontiguousarray(w_out[:, 2 * HD * c : 2 * HD * (c + 1)]).astype(np.float32)
        sinkexp = np.exp(sinks[2 * c : 2 * c + 2].astype(np.float64)).astype(np.float32).reshape(1, 2)
        in_maps.append(
            {
                "x": x2,
                "wq": wq_c,
                "wkv": np.ascontiguousarray(wkv_c),
                "wout": wout_c,
                "cosq": tbl["cosq"],
                "sinq": tbl["sinq"],
                "cosk": tbl["cosk"],
                "sink": tbl["sink"],
                "masks": masks,
                "sinkexp": sinkexp,
                "ident": ident,
            }
        )

    if _cached_nc is None:
        _cached_nc = build_graph()
    nc = _cached_nc

    res = run_bass_kernel_spmd(
        nc, in_maps, core_ids=list(range(NCORE)), trace=TRACE
    )
    LAST_EXEC_NS = res.exec_time_ns
    LAST_RESULT = res
    shards = [res.results[c]["out"] for c in range(NCORE)]
    out = np.concatenate(shards, axis=0).reshape(1, S, DM).astype(np.float32)
    return out


if __name__ == "__main__":
    nc = build_graph()
    print("graph built ok")
